# revision 3
# baseline (speedup 1.0000x reference)
"""nn_ContrastiveMoCoKnnInitByBert — Trainium2 Bass kernel.

K1 (8 cores, K-sharded): cos = liner_q @ feature_queue.T -> [128, 65536]
   bf16 inputs (halves HBM traffic, 2x PE rate), f32 PSUM accumulate,
   f32 cos output.
host: mask positives to -inf (from labels), convert negatives to bf16;
   extract positives and sort them in f32 (tiny: [128, ~2048]).
K2 (8 cores, B-sharded, 16 rows/core): full descending bf16 sort of each
   row's 65536 masked negatives via the normalized bitonic network
   (see ksort.py layout notes).  bf16 cast commutes with sorting, so
   output[i] = bf16(reference_sorted[i]) exactly (rel err <= 2^-9).
host: assemble logits_con exactly like the reference.
"""
import sys

for _p in ("/opt/trn_rl_repo", "/root/.axon_site/_ro/trn_rl_repo"):
    if _p not in sys.path:
        sys.path.insert(0, _p)

import numpy as np
import ml_dtypes
import concourse.bass as bass
import concourse.mybir as mybir
from concourse.bass_utils import run_bass_kernel_spmd

f32 = mybir.dt.float32
bf16 = mybir.dt.bfloat16
NCORES = 8
B, K, C = 128, 65536, 768
KC = K // NCORES          # 8192 cols per core in K1
RC = B // NCORES          # 16 rows per core in K2
T = 0.3

_cache = {}


# ---------------------------------------------------------------- K1: matmul
def build_k1():
    """cos[128, KC] = qT.T @ fT, bf16 operands, f32 accumulate/output."""
    if "k1" in _cache:
        return _cache["k1"]
    nc = bass.Bass()
    qT = nc.declare_dram_parameter("qT", [C, B], bf16, isOutput=False)
    fT = nc.declare_dram_parameter("fT", [C, KC], bf16, isOutput=False)
    cos = nc.declare_dram_parameter("cos", [B, KC], f32, isOutput=True)

    CH = 2048                 # k-chunk width
    NCH = KC // CH            # 4 chunks
    NS = CH // 512            # 4 psum groups per chunk
    NC6 = C // 128            # 6 contraction sub-blocks

    with (
        nc.sbuf_tensor([128, C], bf16) as qsb,
        nc.sbuf_tensor([128, NC6 * CH], bf16) as fz,
        nc.sbuf_tensor([128, CH], f32) as st0,
        nc.sbuf_tensor([128, CH], f32) as st1,
        nc.psum_tensor([128, 512], f32) as ps0,
        nc.psum_tensor([128, 512], f32) as ps1,
        nc.semaphore("dsem") as dsem,
        nc.semaphore("msem") as msem,
        nc.semaphore("asem") as asem,
        nc.semaphore("osem") as osem,
        nc.Block() as block,
    ):
        stg = [st0, st1]
        pss = [ps0, ps1]

        @block.sync
        def _(sync):
            for c6 in range(NC6):
                sync.dma_start(out=qsb[:, c6 * 128:(c6 + 1) * 128],
                               in_=qT[c6 * 128:(c6 + 1) * 128, :]).then_inc(dsem, 16)
            for ci in range(NCH):
                if ci >= 1:
                    sync.wait_ge(asem, NS * ci)           # fz free (chunk ci-1 consumed)
                    sync.dma_start(out=cos[:, (ci - 1) * CH:ci * CH],
                                   in_=stg[(ci - 1) % 2][:, :]).then_inc(osem, 16)
                for c6 in range(NC6):
                    sync.dma_start(
                        out=fz[:, c6 * CH:(c6 + 1) * CH],
                        in_=fT[c6 * 128:(c6 + 1) * 128, ci * CH:(ci + 1) * CH],
                    ).then_inc(dsem, 16)
            sync.wait_ge(asem, NS * NCH)
            sync.dma_start(out=cos[:, (NCH - 1) * CH:NCH * CH],
                           in_=stg[(NCH - 1) % 2][:, :]).then_inc(osem, 16)

        @block.tensor
        def _(tensor):
            g = 0
            for ci in range(NCH):
                tensor.wait_ge(dsem, 16 * (NC6 + NC6 * (ci + 1)))
                for ns in range(NS):
                    if g >= 2:
                        tensor.wait_ge(asem, g - 1)       # psum bank free
                    ps = pss[g % 2]
                    for c6 in range(NC6):
                        ins = nc.tensor.matmul(
                            out=ps[:, :],
                            lhsT=qsb[:, c6 * 128:(c6 + 1) * 128],
                            rhs=fz[:, c6 * CH + ns * 512: c6 * CH + (ns + 1) * 512],
                            start=(c6 == 0), stop=(c6 == NC6 - 1),
                        )
                        if c6 == NC6 - 1:
                            ins.then_inc(msem, 1)
                    g += 1

        @block.scalar
        def _(scalar):
            g = 0
            for ci in range(NCH):
                for ns in range(NS):
                    scalar.wait_ge(msem, g + 1)
                    if ci >= 2 and ns == 0:
                        scalar.wait_ge(osem, 16 * (ci - 1))   # stage tile free
                    nc.scalar.copy(out=stg[ci % 2][:, ns * 512:(ns + 1) * 512],
                                   in_=pss[g % 2][:, :]).then_inc(asem, 1)
                    g += 1

    _cache["k1"] = nc
    return nc


W = 8192                  # elems per partition
Q = 8                     # chunks per row
QP = 16                   # partitions per chunk block (p = q*QP + r)

_cache = {}


def stages_for(n_bits):
    out = []
    for L in range(1, n_bits + 1):
        out.append(("mirror", L))
        for j in range(L - 2, -1, -1):
            out.append(("stride", j))
    return out


def qruns(qs):
    runs = []
    for q in qs:
        if runs and runs[-1][0] + runs[-1][1] == q:
            runs[-1][1] += 1
        else:
            runs.append([q, 1])
    return runs


def plan_sort(bufA, bufB, aux, w, n_bits):
    w_bits = int(np.log2(w))
    steps = []
    src, dst = bufA, bufB
    for kind, p in stages_for(n_bits):
        cross = (kind == "mirror" and p > w_bits) or (kind == "stride" and p >= w_bits)
        if not cross:
            steps.append({"t": "intra", "kind": kind, "p": p, "src": src, "dst": dst})
        else:
            if kind == "mirror":
                b = p - 1 - w_bits
                xm = (1 << (p - w_bits)) - 1
                frev = True
            else:
                b = p - w_bits
                xm = 1 << b
                frev = False
            hi_runs = qruns([q for q in range(Q) if q & (1 << b)])
            steps.append({"t": "cross", "xm": xm, "frev": frev, "hi": hi_runs,
                          "src": src, "dst": dst, "w": w, "aux": aux})
        src, dst = dst, src
    return steps, src


def build_ksort(repeat=1):
    key = ("ks", repeat)
    if key in _cache:
        return _cache[key]
    nc = bass.Bass()
    neg_in = nc.declare_dram_parameter("neg", [RC, K], bf16, isOutput=False)
    sneg = nc.declare_dram_parameter("sneg", [RC, K], bf16, isOutput=True)

    with (
        nc.sbuf_tensor([128, W], bf16) as nA,
        nc.sbuf_tensor([128, W], bf16) as nB,
        nc.sbuf_tensor([128, W], bf16) as nAux,
        nc.semaphore("dsem") as dsem,
        nc.semaphore("xsem") as xsem,
        nc.semaphore("dxsem") as dxsem,
        nc.Block() as block,
    ):
        steps_n, fin_n = plan_sort(nA, nB, nAux, W, 16)
        all_steps = [dict(s) for _ in range(repeat) for s in steps_n]

        xc = 0
        dxc = 0
        for s in all_steps:
            if s["t"] == "cross":
                xc += 1
                s["x_ready"] = xc
                dxc += 16 * Q
                s["dx_aux"] = dxc
                xc += 1
                s["x_min"] = xc
                dxc += 16 * len(s["hi"])
                s["dx_gather"] = dxc
        xc += 1
        x_final = xc

        @block.sync
        def _(sync):
            sync.dma_start(out=nA[:, :],
                           in_=neg_in.rearrange("r (q f) -> q r f", q=Q)).then_inc(dsem, 16)
            for s in all_steps:
                if s["t"] != "cross":
                    continue
                src, dst, aux, xm = s["src"], s["dst"], s["aux"], s["xm"]
                sync.wait_ge(xsem, s["x_ready"])
                for q in range(Q):
                    sync.dma_start(out=aux[q * QP:(q + 1) * QP, :],
                                   in_=src[(q ^ xm) * QP:((q ^ xm) + 1) * QP, :]
                                   ).then_inc(dxsem, 16)
                sync.wait_ge(xsem, s["x_min"])
                for q0, ln in s["hi"]:
                    sync.dma_start(out=dst[q0 * QP:(q0 + ln) * QP, :],
                                   in_=src[q0 * QP:(q0 + ln) * QP, :]).then_inc(dxsem, 16)
            sync.wait_ge(xsem, x_final)
            sync.dma_start(out=sneg.rearrange("r (q f) -> q r f", q=Q),
                           in_=fin_n[:, :]).then_inc(dsem, 16)

        @block.vector
        def _(vector):
            vector.wait_ge(dsem, 16)
            mx, mn = mybir.AluOpType.max, mybir.AluOpType.min
            for s in all_steps:
                src, dst = s["src"], s["dst"]
                if s["t"] == "intra":
                    kind, p = s["kind"], s["p"]
                    if kind == "mirror":
                        m = 1 << p
                        h = m // 2
                        rs = src.rearrange("p (b m) -> p b m", m=m)
                        rd = dst.rearrange("p (b m) -> p b m", m=m)
                        nc.vector.tensor_tensor(out=rd[:, :, 0:h], in0=rs[:, :, 0:h],
                                                in1=rs[:, :, m - 1:h - 1:-1], op=mx)
                        nc.vector.tensor_tensor(out=rd[:, :, h:m], in0=rs[:, :, h - 1::-1],
                                                in1=rs[:, :, h:m], op=mn)
                    elif p == 0:
                        rs = src.rearrange("p (b h) -> p b h", h=2)
                        rd = dst.rearrange("p (b h) -> p b h", h=2)
                        nc.vector.tensor_tensor(out=rd[:, :, 0], in0=rs[:, :, 0],
                                                in1=rs[:, :, 1], op=mx)
                        nc.vector.tensor_tensor(out=rd[:, :, 1], in0=rs[:, :, 0],
                                                in1=rs[:, :, 1], op=mn)
                    else:
                        st = 1 << p
                        rs = src.rearrange("p (b h x) -> p b h x", h=2, x=st)
                        rd = dst.rearrange("p (b h x) -> p b h x", h=2, x=st)
                        nc.vector.tensor_tensor(out=rd[:, :, 0, :], in0=rs[:, :, 0, :],
                                                in1=rs[:, :, 1, :], op=mx)
                        nc.vector.tensor_tensor(out=rd[:, :, 1, :], in0=rs[:, :, 0, :],
                                                in1=rs[:, :, 1, :], op=mn)
                else:
                    aux, w = s["aux"], s["w"]
                    nc.vector.engine_nop().then_inc(xsem, 1)
                    vector.wait_ge(dxsem, s["dx_aux"])
                    in1 = aux[:, w - 1::-1] if s["frev"] else aux[:, 0:w]
                    nc.vector.tensor_tensor(out=dst[:, :], in0=src[:, :], in1=in1, op=mx)
                    nc.vector.tensor_tensor(out=src[:, :], in0=src[:, :], in1=in1,
                                            op=mn).then_inc(xsem, 1)
                    vector.wait_ge(dxsem, s["dx_gather"])
            nc.vector.engine_nop().then_inc(xsem, 1)

    _cache[key] = nc
    return nc


# ----------------------------------------------------------------- host side
def kernel(liner_q, feature_queue, label_q, label_queue, top_k):
    liner_q = np.ascontiguousarray(np.asarray(liner_q, dtype=np.float32))
    F = np.asarray(feature_queue, dtype=np.float32)
    lq = np.asarray(label_q).astype(np.int64)
    lqueue = np.asarray(label_queue).astype(np.int64)
    top_k = int(np.asarray(top_k))

    cores = list(range(NCORES))

    # ---------------- K1: cos = Q @ F^T, K-sharded, bf16 inputs
    qT = np.ascontiguousarray(liner_q.T).astype(ml_dtypes.bfloat16)    # [C, B]
    FT = np.ascontiguousarray(F.T).astype(ml_dtypes.bfloat16)          # [C, K]
    nc1 = build_k1()
    in_maps1 = [{"qT": qT, "fT": np.ascontiguousarray(FT[:, c * KC:(c + 1) * KC])}
                for c in cores]
    res1 = run_bass_kernel_spmd(nc1, in_maps1, core_ids=cores)
    cos = np.concatenate([res1.results[c]["cos"] for c in cores], axis=1)  # [B, K] f32

    # ---------------- host mask; negatives -> bf16
    mask = lq[:, None] == lqueue[None, :]                      # [B, K]
    cnt = mask.sum(-1)
    pos_min = int(cnt.min())
    neg_min = int(K - cnt.max())
    assert pos_min > 0 and neg_min > 0

    neg = np.where(mask, np.float32(-np.inf), cos)             # [B, K]
    neg_bf = neg.astype(ml_dtypes.bfloat16)

    # ---------------- K2: descending bf16 sort of negatives, B-sharded
    nc2 = build_ksort()
    in_maps2 = [{"neg": neg_bf[c * RC:(c + 1) * RC]} for c in cores]
    res2 = run_bass_kernel_spmd(nc2, in_maps2, core_ids=cores)
    sneg = np.concatenate([res2.results[c]["sneg"] for c in cores],
                          axis=0).astype(np.float32)           # [B, K]

    # ---------------- host: positives (tiny) in f32
    posw = int(cnt.max())
    pos_pad = np.full((B, posw), -np.inf, dtype=np.float32)
    rows, cols = np.nonzero(mask)
    within = np.arange(rows.size) - np.repeat(
        np.concatenate([[0], np.cumsum(cnt)[:-1]]), cnt)
    pos_pad[rows, within] = cos[rows, cols]
    spos = -np.sort(-pos_pad, axis=-1)[:, :pos_min]            # [B, pos_min]

    # ---------------- host assembly (matches reference exactly)
    tk = min(top_k, pos_min)
    pos_cat = np.concatenate([spos[:, :tk], spos[:, pos_min - 1:pos_min]], axis=1)
    reps = pos_cat.shape[1]
    Tf = np.float32(T)
    pos_scaled = (pos_cat / Tf).astype(np.float32)
    neg_scaled = (sneg[:, :neg_min] / Tf).astype(np.float32)

    out = np.empty((B * reps, 1 + neg_min), dtype=np.float32)
    out3 = out.reshape(B, reps, 1 + neg_min)
    out3[:, :, 0] = pos_scaled
    out3[:, :, 1:] = neg_scaled[:, None, :]
    return out


# revision 7
# speedup vs baseline: 1.0686x; 1.0686x over previous
"""nn_ContrastiveMoCoKnnInitByBert — Trainium2 Bass kernel.

K1 (8 cores, K-sharded): cos = liner_q @ feature_queue.T -> [128, 65536]
   bf16 inputs (halves HBM traffic, 2x PE rate), f32 PSUM accumulate,
   f32 cos output.
host: mask positives to -inf (from labels), convert negatives to bf16;
   extract positives and sort them in f32 (tiny: [128, ~2048]).
K2 (8 cores, B-sharded, 16 rows/core): full descending bf16 sort of each
   row's 65536 masked negatives via the normalized bitonic network
   (see ksort.py layout notes).  bf16 cast commutes with sorting, so
   output[i] = bf16(reference_sorted[i]) exactly (rel err <= 2^-9).
host: assemble logits_con exactly like the reference.
"""
import sys

for _p in ("/opt/trn_rl_repo", "/root/.axon_site/_ro/trn_rl_repo"):
    if _p not in sys.path:
        sys.path.insert(0, _p)

import numpy as np
import ml_dtypes
import concourse.bass as bass
import concourse.mybir as mybir
from concourse.bass_utils import run_bass_kernel_spmd

f32 = mybir.dt.float32
bf16 = mybir.dt.bfloat16
NCORES = 8
B, K, C = 128, 65536, 768
KC = K // NCORES          # 8192 cols per core in K1
RC = B // NCORES          # 16 rows per core in K2
T = 0.3

_cache = {}


# ---------------------------------------------------------------- K1: matmul
def build_k1():
    """cos[128, KC] = qT.T @ fT, bf16 operands, f32 accumulate/output."""
    if "k1" in _cache:
        return _cache["k1"]
    nc = bass.Bass()
    qT = nc.declare_dram_parameter("qT", [C, B], bf16, isOutput=False)
    fT = nc.declare_dram_parameter("fT", [C, KC], bf16, isOutput=False)
    cos = nc.declare_dram_parameter("cos", [B, KC], f32, isOutput=True)

    CH = 2048                 # k-chunk width
    NCH = KC // CH            # 4 chunks
    NS = CH // 512            # 4 psum groups per chunk
    NC6 = C // 128            # 6 contraction sub-blocks

    with (
        nc.sbuf_tensor([128, C], bf16) as qsb,
        nc.sbuf_tensor([128, NC6 * CH], bf16) as fz,
        nc.sbuf_tensor([128, CH], f32) as st0,
        nc.sbuf_tensor([128, CH], f32) as st1,
        nc.psum_tensor([128, 512], f32) as ps0,
        nc.psum_tensor([128, 512], f32) as ps1,
        nc.semaphore("dsem") as dsem,
        nc.semaphore("msem") as msem,
        nc.semaphore("asem") as asem,
        nc.semaphore("osem") as osem,
        nc.Block() as block,
    ):
        stg = [st0, st1]
        pss = [ps0, ps1]

        @block.sync
        def _(sync):
            for c6 in range(NC6):
                sync.dma_start(out=qsb[:, c6 * 128:(c6 + 1) * 128],
                               in_=qT[c6 * 128:(c6 + 1) * 128, :]).then_inc(dsem, 16)
            for ci in range(NCH):
                if ci >= 1:
                    sync.wait_ge(asem, NS * ci)           # fz free (chunk ci-1 consumed)
                    sync.dma_start(out=cos[:, (ci - 1) * CH:ci * CH],
                                   in_=stg[(ci - 1) % 2][:, :]).then_inc(osem, 16)
                for c6 in range(NC6):
                    sync.dma_start(
                        out=fz[:, c6 * CH:(c6 + 1) * CH],
                        in_=fT[c6 * 128:(c6 + 1) * 128, ci * CH:(ci + 1) * CH],
                    ).then_inc(dsem, 16)
            sync.wait_ge(asem, NS * NCH)
            sync.dma_start(out=cos[:, (NCH - 1) * CH:NCH * CH],
                           in_=stg[(NCH - 1) % 2][:, :]).then_inc(osem, 16)

        @block.tensor
        def _(tensor):
            g = 0
            for ci in range(NCH):
                tensor.wait_ge(dsem, 16 * (NC6 + NC6 * (ci + 1)))
                for ns in range(NS):
                    if g >= 2:
                        tensor.wait_ge(asem, g - 1)       # psum bank free
                    ps = pss[g % 2]
                    for c6 in range(NC6):
                        ins = nc.tensor.matmul(
                            out=ps[:, :],
                            lhsT=qsb[:, c6 * 128:(c6 + 1) * 128],
                            rhs=fz[:, c6 * CH + ns * 512: c6 * CH + (ns + 1) * 512],
                            start=(c6 == 0), stop=(c6 == NC6 - 1),
                        )
                        if c6 == NC6 - 1:
                            ins.then_inc(msem, 1)
                    g += 1

        @block.scalar
        def _(scalar):
            g = 0
            for ci in range(NCH):
                for ns in range(NS):
                    scalar.wait_ge(msem, g + 1)
                    if ci >= 2 and ns == 0:
                        scalar.wait_ge(osem, 16 * (ci - 1))   # stage tile free
                    nc.scalar.copy(out=stg[ci % 2][:, ns * 512:(ns + 1) * 512],
                                   in_=pss[g % 2][:, :]).then_inc(asem, 1)
                    g += 1

    _cache["k1"] = nc
    return nc


W = 8192                  # elems per partition
Q = 8                     # chunks per row
QP = 16                   # partitions per chunk block (p = q*QP + r)

_cache = {}


def stages_for(n_bits):
    out = []
    for L in range(1, n_bits + 1):
        out.append(("mirror", L))
        for j in range(L - 2, -1, -1):
            out.append(("stride", j))
    return out


def qruns(qs):
    runs = []
    for q in qs:
        if runs and runs[-1][0] + runs[-1][1] == q:
            runs[-1][1] += 1
        else:
            runs.append([q, 1])
    return runs


def plan_sort(bufA, bufB, aux, w, n_bits):
    w_bits = int(np.log2(w))
    steps = []
    src, dst = bufA, bufB
    for kind, p in stages_for(n_bits):
        cross = (kind == "mirror" and p > w_bits) or (kind == "stride" and p >= w_bits)
        if not cross:
            steps.append({"t": "intra", "kind": kind, "p": p, "src": src, "dst": dst})
        else:
            if kind == "mirror":
                b = p - 1 - w_bits
                xm = (1 << (p - w_bits)) - 1
                frev = True
            else:
                b = p - w_bits
                xm = 1 << b
                frev = False
            hi_runs = qruns([q for q in range(Q) if q & (1 << b)])
            steps.append({"t": "cross", "xm": xm, "frev": frev, "hi": hi_runs,
                          "src": src, "dst": dst, "w": w, "aux": aux})
        src, dst = dst, src
    return steps, src


def build_ksort(repeat=1):
    key = ("ks", repeat)
    if key in _cache:
        return _cache[key]
    nc = bass.Bass()
    neg_in = nc.declare_dram_parameter("neg", [RC, K], bf16, isOutput=False)
    sneg = nc.declare_dram_parameter("sneg", [RC, K], bf16, isOutput=True)

    with (
        nc.sbuf_tensor([128, W], bf16) as nA,
        nc.sbuf_tensor([128, W], bf16) as nB,
        nc.sbuf_tensor([128, W], bf16) as nAux,
        nc.semaphore("dsem") as dsem,
        nc.semaphore("xsem") as xsem,
        nc.semaphore("dxsem") as dxsem,
        nc.Block() as block,
    ):
        steps_n, fin_n = plan_sort(nA, nB, nAux, W, 16)
        all_steps = [dict(s) for _ in range(repeat) for s in steps_n]

        H = W // 2
        xc = 0
        dxc = 0
        for s in all_steps:
            if s["t"] == "cross":
                # halves pipelined: aux half0, aux half1; compute waits per half
                xc += 1
                s["x_ready"] = xc
                dxc += 16 * Q
                s["dx_aux0"] = dxc          # first-needed aux half landed
                dxc += 16 * Q
                s["dx_aux1"] = dxc          # second aux half landed
                xc += 1
                s["x_min0"] = xc            # compute half 0 done (max+min)
                xc += 1
                s["x_min1"] = xc            # compute half 1 done
                dxc += 16 * len(s["hi"]) * 2
                s["dx_gather"] = dxc
        xc += 1
        x_final = xc

        @block.sync
        def _(sync):
            sync.dma_start(out=nA[:, :],
                           in_=neg_in.rearrange("r (q f) -> q r f", q=Q)).then_inc(dsem, 16)
            for s in all_steps:
                if s["t"] != "cross":
                    continue
                src, dst, aux, xm = s["src"], s["dst"], s["aux"], s["xm"]
                # For frev (mirror) stages the compute on out-half h reads the
                # OTHER aux half reversed, so ship aux halves in reversed order.
                halves = (1, 0) if s["frev"] else (0, 1)
                sync.wait_ge(xsem, s["x_ready"])
                for h in halves:
                    lo, hi = h * H, (h + 1) * H
                    for q in range(Q):
                        sync.dma_start(out=aux[q * QP:(q + 1) * QP, lo:hi],
                                       in_=src[(q ^ xm) * QP:((q ^ xm) + 1) * QP, lo:hi]
                                       ).then_inc(dxsem, 16)
                for h in range(2):
                    lo, hi = h * H, (h + 1) * H
                    sync.wait_ge(xsem, s["x_min0"] + h)
                    for q0, ln in s["hi"]:
                        sync.dma_start(out=dst[q0 * QP:(q0 + ln) * QP, lo:hi],
                                       in_=src[q0 * QP:(q0 + ln) * QP, lo:hi]
                                       ).then_inc(dxsem, 16)
            sync.wait_ge(xsem, x_final)
            sync.dma_start(out=sneg.rearrange("r (q f) -> q r f", q=Q),
                           in_=fin_n[:, :]).then_inc(dsem, 16)

        @block.vector
        def _(vector):
            vector.wait_ge(dsem, 16)
            mx, mn = mybir.AluOpType.max, mybir.AluOpType.min
            for s in all_steps:
                src, dst = s["src"], s["dst"]
                if s["t"] == "intra":
                    kind, p = s["kind"], s["p"]
                    if kind == "mirror":
                        m = 1 << p
                        h = m // 2
                        rs = src.rearrange("p (b m) -> p b m", m=m)
                        rd = dst.rearrange("p (b m) -> p b m", m=m)
                        nc.vector.tensor_tensor(out=rd[:, :, 0:h], in0=rs[:, :, 0:h],
                                                in1=rs[:, :, m - 1:h - 1:-1], op=mx)
                        nc.vector.tensor_tensor(out=rd[:, :, h:m], in0=rs[:, :, h - 1::-1],
                                                in1=rs[:, :, h:m], op=mn)
                    elif p == 0:
                        rs = src.rearrange("p (b h) -> p b h", h=2)
                        rd = dst.rearrange("p (b h) -> p b h", h=2)
                        nc.vector.tensor_tensor(out=rd[:, :, 0], in0=rs[:, :, 0],
                                                in1=rs[:, :, 1], op=mx)
                        nc.vector.tensor_tensor(out=rd[:, :, 1], in0=rs[:, :, 0],
                                                in1=rs[:, :, 1], op=mn)
                    else:
                        st = 1 << p
                        rs = src.rearrange("p (b h x) -> p b h x", h=2, x=st)
                        rd = dst.rearrange("p (b h x) -> p b h x", h=2, x=st)
                        nc.vector.tensor_tensor(out=rd[:, :, 0, :], in0=rs[:, :, 0, :],
                                                in1=rs[:, :, 1, :], op=mx)
                        nc.vector.tensor_tensor(out=rd[:, :, 1, :], in0=rs[:, :, 0, :],
                                                in1=rs[:, :, 1, :], op=mn)
                else:
                    aux, w = s["aux"], s["w"]
                    h2 = w // 2
                    nc.vector.engine_nop().then_inc(xsem, 1)
                    for h in range(2):
                        lo, hi = h * h2, (h + 1) * h2
                        vector.wait_ge(dxsem, s["dx_aux0"] if h == 0 else s["dx_aux1"])
                        if s["frev"]:
                            in1 = aux[:, w - 1 - lo:w - 1 - hi:-1] if hi < w \
                                else aux[:, w - 1 - lo::-1]
                        else:
                            in1 = aux[:, lo:hi]
                        nc.vector.tensor_tensor(out=dst[:, lo:hi], in0=src[:, lo:hi],
                                                in1=in1, op=mx)
                        nc.vector.tensor_tensor(out=src[:, lo:hi], in0=src[:, lo:hi],
                                                in1=in1, op=mn).then_inc(xsem, 1)
                    vector.wait_ge(dxsem, s["dx_gather"])
            nc.vector.engine_nop().then_inc(xsem, 1)

    _cache[key] = nc
    return nc


# ----------------------------------------------------------------- host side
def kernel(liner_q, feature_queue, label_q, label_queue, top_k):
    liner_q = np.ascontiguousarray(np.asarray(liner_q, dtype=np.float32))
    F = np.asarray(feature_queue, dtype=np.float32)
    lq = np.asarray(label_q).astype(np.int64)
    lqueue = np.asarray(label_queue).astype(np.int64)
    top_k = int(np.asarray(top_k))

    cores = list(range(NCORES))

    # ---------------- K1: cos = Q @ F^T, K-sharded, bf16 inputs
    qT = np.ascontiguousarray(liner_q.T).astype(ml_dtypes.bfloat16)    # [C, B]
    FT = np.ascontiguousarray(F.T).astype(ml_dtypes.bfloat16)          # [C, K]
    nc1 = build_k1()
    in_maps1 = [{"qT": qT, "fT": np.ascontiguousarray(FT[:, c * KC:(c + 1) * KC])}
                for c in cores]
    res1 = run_bass_kernel_spmd(nc1, in_maps1, core_ids=cores)
    cos = np.concatenate([res1.results[c]["cos"] for c in cores], axis=1)  # [B, K] f32

    # ---------------- host mask; negatives -> bf16
    mask = lq[:, None] == lqueue[None, :]                      # [B, K]
    cnt = mask.sum(-1)
    pos_min = int(cnt.min())
    neg_min = int(K - cnt.max())
    assert pos_min > 0 and neg_min > 0

    neg = np.where(mask, np.float32(-np.inf), cos)             # [B, K]
    neg_bf = neg.astype(ml_dtypes.bfloat16)

    # ---------------- K2: descending bf16 sort of negatives, B-sharded
    nc2 = build_ksort()
    in_maps2 = [{"neg": neg_bf[c * RC:(c + 1) * RC]} for c in cores]
    res2 = run_bass_kernel_spmd(nc2, in_maps2, core_ids=cores)
    sneg = np.concatenate([res2.results[c]["sneg"] for c in cores],
                          axis=0).astype(np.float32)           # [B, K]

    # ---------------- host: positives (tiny) in f32
    posw = int(cnt.max())
    pos_pad = np.full((B, posw), -np.inf, dtype=np.float32)
    rows, cols = np.nonzero(mask)
    within = np.arange(rows.size) - np.repeat(
        np.concatenate([[0], np.cumsum(cnt)[:-1]]), cnt)
    pos_pad[rows, within] = cos[rows, cols]
    spos = -np.sort(-pos_pad, axis=-1)[:, :pos_min]            # [B, pos_min]

    # ---------------- host assembly (matches reference exactly)
    tk = min(top_k, pos_min)
    pos_cat = np.concatenate([spos[:, :tk], spos[:, pos_min - 1:pos_min]], axis=1)
    reps = pos_cat.shape[1]
    Tf = np.float32(T)
    pos_scaled = (pos_cat / Tf).astype(np.float32)
    neg_scaled = (sneg[:, :neg_min] / Tf).astype(np.float32)

    out = np.empty((B * reps, 1 + neg_min), dtype=np.float32)
    out3 = out.reshape(B, reps, 1 + neg_min)
    out3[:, :, 0] = pos_scaled
    out3[:, :, 1:] = neg_scaled[:, None, :]
    return out


# revision 9
# speedup vs baseline: 1.0913x; 1.0213x over previous
"""nn_ContrastiveMoCoKnnInitByBert — Trainium2 Bass kernel.

K1 (8 cores, K-sharded): cos = liner_q @ feature_queue.T -> [128, 65536]
   bf16 inputs (halves HBM traffic, 2x PE rate), f32 PSUM accumulate,
   f32 cos output.
host: mask positives to -inf (from labels), convert negatives to bf16;
   extract positives and sort them in f32 (tiny: [128, ~2048]).
K2 (8 cores, B-sharded, 16 rows/core): full descending bf16 sort of each
   row's 65536 masked negatives via the normalized bitonic network
   (see ksort.py layout notes).  bf16 cast commutes with sorting, so
   output[i] = bf16(reference_sorted[i]) exactly (rel err <= 2^-9).
host: assemble logits_con exactly like the reference.
"""
import sys

for _p in ("/opt/trn_rl_repo", "/root/.axon_site/_ro/trn_rl_repo"):
    if _p not in sys.path:
        sys.path.insert(0, _p)

import numpy as np
import ml_dtypes
import concourse.bass as bass
import concourse.mybir as mybir
from concourse.bass_utils import run_bass_kernel_spmd

f32 = mybir.dt.float32
bf16 = mybir.dt.bfloat16
NCORES = 8
B, K, C = 128, 65536, 768
KC = K // NCORES          # 8192 cols per core in K1
RC = B // NCORES          # 16 rows per core in K2
T = 0.3

_cache = {}


# ---------------------------------------------------------------- K1: matmul
def build_k1():
    """cos[128, KC] = qT.T @ fT, bf16 operands, f32 accumulate/output."""
    if "k1" in _cache:
        return _cache["k1"]
    nc = bass.Bass()
    qT = nc.declare_dram_parameter("qT", [C, B], bf16, isOutput=False)
    fT = nc.declare_dram_parameter("fT", [C, KC], bf16, isOutput=False)
    cos = nc.declare_dram_parameter("cos", [B, KC], f32, isOutput=True)

    CH = 2048                 # k-chunk width
    NCH = KC // CH            # 4 chunks
    NS = CH // 512            # 4 psum groups per chunk
    NC6 = C // 128            # 6 contraction sub-blocks

    with (
        nc.sbuf_tensor([128, C], bf16) as qsb,
        nc.sbuf_tensor([128, NC6 * CH], bf16) as fz,
        nc.sbuf_tensor([128, CH], f32) as st0,
        nc.sbuf_tensor([128, CH], f32) as st1,
        nc.psum_tensor([128, 512], f32) as ps0,
        nc.psum_tensor([128, 512], f32) as ps1,
        nc.semaphore("dsem") as dsem,
        nc.semaphore("msem") as msem,
        nc.semaphore("asem") as asem,
        nc.semaphore("osem") as osem,
        nc.Block() as block,
    ):
        stg = [st0, st1]
        pss = [ps0, ps1]

        @block.sync
        def _(sync):
            for c6 in range(NC6):
                sync.dma_start(out=qsb[:, c6 * 128:(c6 + 1) * 128],
                               in_=qT[c6 * 128:(c6 + 1) * 128, :]).then_inc(dsem, 16)
            for ci in range(NCH):
                if ci >= 1:
                    sync.wait_ge(asem, NS * ci)           # fz free (chunk ci-1 consumed)
                    sync.dma_start(out=cos[:, (ci - 1) * CH:ci * CH],
                                   in_=stg[(ci - 1) % 2][:, :]).then_inc(osem, 16)
                for c6 in range(NC6):
                    sync.dma_start(
                        out=fz[:, c6 * CH:(c6 + 1) * CH],
                        in_=fT[c6 * 128:(c6 + 1) * 128, ci * CH:(ci + 1) * CH],
                    ).then_inc(dsem, 16)
            sync.wait_ge(asem, NS * NCH)
            sync.dma_start(out=cos[:, (NCH - 1) * CH:NCH * CH],
                           in_=stg[(NCH - 1) % 2][:, :]).then_inc(osem, 16)

        @block.tensor
        def _(tensor):
            g = 0
            for ci in range(NCH):
                tensor.wait_ge(dsem, 16 * (NC6 + NC6 * (ci + 1)))
                for ns in range(NS):
                    if g >= 2:
                        tensor.wait_ge(asem, g - 1)       # psum bank free
                    ps = pss[g % 2]
                    for c6 in range(NC6):
                        ins = nc.tensor.matmul(
                            out=ps[:, :],
                            lhsT=qsb[:, c6 * 128:(c6 + 1) * 128],
                            rhs=fz[:, c6 * CH + ns * 512: c6 * CH + (ns + 1) * 512],
                            start=(c6 == 0), stop=(c6 == NC6 - 1),
                        )
                        if c6 == NC6 - 1:
                            ins.then_inc(msem, 1)
                    g += 1

        @block.scalar
        def _(scalar):
            g = 0
            for ci in range(NCH):
                for ns in range(NS):
                    scalar.wait_ge(msem, g + 1)
                    if ci >= 2 and ns == 0:
                        scalar.wait_ge(osem, 16 * (ci - 1))   # stage tile free
                    nc.scalar.copy(out=stg[ci % 2][:, ns * 512:(ns + 1) * 512],
                                   in_=pss[g % 2][:, :]).then_inc(asem, 1)
                    g += 1

    _cache["k1"] = nc
    return nc


W = 8192                  # elems per partition
Q = 8                     # chunks per row
QP = 16                   # partitions per chunk block (p = q*QP + r)

_cache = {}


def stages_for(n_bits):
    out = []
    for L in range(1, n_bits + 1):
        out.append(("mirror", L))
        for j in range(L - 2, -1, -1):
            out.append(("stride", j))
    return out


def qruns(qs):
    runs = []
    for q in qs:
        if runs and runs[-1][0] + runs[-1][1] == q:
            runs[-1][1] += 1
        else:
            runs.append([q, 1])
    return runs


def plan_sort(bufA, bufB, aux, w, n_bits):
    w_bits = int(np.log2(w))
    steps = []
    src, dst = bufA, bufB
    for kind, p in stages_for(n_bits):
        cross = (kind == "mirror" and p > w_bits) or (kind == "stride" and p >= w_bits)
        if not cross:
            steps.append({"t": "intra", "kind": kind, "p": p, "src": src, "dst": dst})
        else:
            if kind == "mirror":
                b = p - 1 - w_bits
                xm = (1 << (p - w_bits)) - 1
                frev = True
            else:
                b = p - w_bits
                xm = 1 << b
                frev = False
            hi_runs = qruns([q for q in range(Q) if q & (1 << b)])
            steps.append({"t": "cross", "xm": xm, "frev": frev, "hi": hi_runs,
                          "src": src, "dst": dst, "w": w, "aux": aux})
        src, dst = dst, src
    return steps, src


def build_ksort(repeat=1, dual=None):
    key = ("ks", repeat, dual)
    if key in _cache:
        return _cache[key]
    nc = bass.Bass()
    neg_in = nc.declare_dram_parameter("neg", [RC, K], bf16, isOutput=False)
    sneg = nc.declare_dram_parameter("sneg", [RC, K], bf16, isOutput=True)

    H = W // 2

    with (
        nc.sbuf_tensor([128, W], bf16) as nA,
        nc.sbuf_tensor([128, W], bf16) as nB,
        nc.sbuf_tensor([128, W], bf16) as nAux,
        nc.semaphore("dsem") as dsem,
        nc.semaphore("xsem") as xsem,
        nc.semaphore("dxA") as dxA,
        nc.semaphore("dxB") as dxB,
        nc.semaphore("gsem") as gsem,
        nc.Block() as block,
    ):
        steps_n, fin_n = plan_sort(nA, nB, nAux, W, 16)
        all_steps = [dict(s) for _ in range(repeat) for s in steps_n]

        # annotate cross steps with xsem / gsem targets
        ci = 0
        gcum = 0
        for s in all_steps:
            if s["t"] == "cross":
                s["ci"] = ci
                s["x_ready"] = 3 * ci + 1
                s["x_min0"] = 3 * ci + 2
                s["x_min1"] = 3 * ci + 3
                gcum += (64 if dual else 32) * len(s["hi"])
                s["g_done"] = gcum
                ci += 1
        ncross = ci
        x_final = 3 * ncross + 1

        def emit_dma_engine(eng, qlist, gather_sel):
            """Cross-stage DMA program for one engine.

            qlist: aux q-block indices this engine ships.
            gather_sel: 0 -> first half of each hi-run, 1 -> second half.
            """
            for s in all_steps:
                if s["t"] != "cross":
                    continue
                src, dst, aux, xm = s["src"], s["dst"], s["aux"], s["xm"]
                halves = (1, 0) if s["frev"] else (0, 1)
                eng.wait_ge(xsem, s["x_ready"])
                for hidx, h in enumerate(halves):
                    lo, hi = h * H, (h + 1) * H
                    sem = dxA if hidx == 0 else dxB
                    for q in qlist:
                        eng.dma_start(out=aux[q * QP:(q + 1) * QP, lo:hi],
                                      in_=src[(q ^ xm) * QP:((q ^ xm) + 1) * QP, lo:hi]
                                      ).then_inc(sem, 16)
                for h in range(2):
                    lo, hi = h * H, (h + 1) * H
                    eng.wait_ge(xsem, s["x_min0"] + h)
                    for q0, ln in s["hi"]:
                        p0 = q0 * QP
                        pn = ln * QP
                        if gather_sel is None:
                            eng.dma_start(out=dst[p0:p0 + pn, lo:hi],
                                          in_=src[p0:p0 + pn, lo:hi]
                                          ).then_inc(gsem, 16)
                        else:
                            # split the run's partitions between the two engines
                            half_p = pn // 2
                            base = p0 + gather_sel * half_p
                            eng.dma_start(out=dst[base:base + half_p, lo:hi],
                                          in_=src[base:base + half_p, lo:hi]
                                          ).then_inc(gsem, 16)

        qs1 = range(0, Q // 2) if dual else range(Q)
        qs2 = range(Q // 2, Q)

        @block.sync
        def _(sync):
            sync.dma_start(out=nA[:, :],
                           in_=neg_in.rearrange("r (q f) -> q r f", q=Q)).then_inc(dsem, 16)
            emit_dma_engine(sync, qs1, 0 if dual else None)
            sync.wait_ge(xsem, x_final)
            sync.dma_start(out=sneg.rearrange("r (q f) -> q r f", q=Q),
                           in_=fin_n[:, :]).then_inc(dsem, 16)

        if dual == "scalar":
            @block.scalar
            def _(scalar):
                emit_dma_engine(scalar, qs2, 1)
        elif dual == "gpsimd":
            @block.gpsimd
            def _(gp):
                emit_dma_engine(gp, qs2, 1)

        @block.vector
        def _(vector):
            vector.wait_ge(dsem, 16)
            mx, mn = mybir.AluOpType.max, mybir.AluOpType.min
            for s in all_steps:
                src, dst = s["src"], s["dst"]
                if s["t"] == "intra":
                    kind, p = s["kind"], s["p"]
                    if kind == "mirror":
                        m = 1 << p
                        h = m // 2
                        rs = src.rearrange("p (b m) -> p b m", m=m)
                        rd = dst.rearrange("p (b m) -> p b m", m=m)
                        nc.vector.tensor_tensor(out=rd[:, :, 0:h], in0=rs[:, :, 0:h],
                                                in1=rs[:, :, m - 1:h - 1:-1], op=mx)
                        nc.vector.tensor_tensor(out=rd[:, :, h:m], in0=rs[:, :, h - 1::-1],
                                                in1=rs[:, :, h:m], op=mn)
                    elif p == 0:
                        rs = src.rearrange("p (b h) -> p b h", h=2)
                        rd = dst.rearrange("p (b h) -> p b h", h=2)
                        nc.vector.tensor_tensor(out=rd[:, :, 0], in0=rs[:, :, 0],
                                                in1=rs[:, :, 1], op=mx)
                        nc.vector.tensor_tensor(out=rd[:, :, 1], in0=rs[:, :, 0],
                                                in1=rs[:, :, 1], op=mn)
                    else:
                        st = 1 << p
                        rs = src.rearrange("p (b h x) -> p b h x", h=2, x=st)
                        rd = dst.rearrange("p (b h x) -> p b h x", h=2, x=st)
                        nc.vector.tensor_tensor(out=rd[:, :, 0, :], in0=rs[:, :, 0, :],
                                                in1=rs[:, :, 1, :], op=mx)
                        nc.vector.tensor_tensor(out=rd[:, :, 1, :], in0=rs[:, :, 0, :],
                                                in1=rs[:, :, 1, :], op=mn)
                else:
                    aux, w = s["aux"], s["w"]
                    nc.vector.engine_nop().then_inc(xsem, 1)
                    for h in range(2):
                        lo, hi = h * H, (h + 1) * H
                        sem = dxA if h == 0 else dxB
                        vector.wait_ge(sem, 128 * (s["ci"] + 1))
                        if s["frev"]:
                            in1 = aux[:, w - 1 - lo:w - 1 - hi:-1] if hi < w \
                                else aux[:, w - 1 - lo::-1]
                        else:
                            in1 = aux[:, lo:hi]
                        nc.vector.tensor_tensor(out=dst[:, lo:hi], in0=src[:, lo:hi],
                                                in1=in1, op=mx)
                        nc.vector.tensor_tensor(out=src[:, lo:hi], in0=src[:, lo:hi],
                                                in1=in1, op=mn).then_inc(xsem, 1)
                    vector.wait_ge(gsem, s["g_done"])
            nc.vector.engine_nop().then_inc(xsem, 1)

    _cache[key] = nc
    return nc


# ----------------------------------------------------------------- host side
def kernel(liner_q, feature_queue, label_q, label_queue, top_k):
    liner_q = np.ascontiguousarray(np.asarray(liner_q, dtype=np.float32))
    F = np.asarray(feature_queue, dtype=np.float32)
    lq = np.asarray(label_q).astype(np.int64)
    lqueue = np.asarray(label_queue).astype(np.int64)
    top_k = int(np.asarray(top_k))

    cores = list(range(NCORES))

    # ---------------- K1: cos = Q @ F^T, K-sharded, bf16 inputs
    qT = np.ascontiguousarray(liner_q.T).astype(ml_dtypes.bfloat16)    # [C, B]
    FT = np.ascontiguousarray(F.T).astype(ml_dtypes.bfloat16)          # [C, K]
    nc1 = build_k1()
    in_maps1 = [{"qT": qT, "fT": np.ascontiguousarray(FT[:, c * KC:(c + 1) * KC])}
                for c in cores]
    res1 = run_bass_kernel_spmd(nc1, in_maps1, core_ids=cores)
    cos = np.concatenate([res1.results[c]["cos"] for c in cores], axis=1)  # [B, K] f32

    # ---------------- host mask; negatives -> bf16
    mask = lq[:, None] == lqueue[None, :]                      # [B, K]
    cnt = mask.sum(-1)
    pos_min = int(cnt.min())
    neg_min = int(K - cnt.max())
    assert pos_min > 0 and neg_min > 0

    neg = np.where(mask, np.float32(-np.inf), cos)             # [B, K]
    neg_bf = neg.astype(ml_dtypes.bfloat16)

    # ---------------- K2: descending bf16 sort of negatives, B-sharded
    nc2 = build_ksort()
    in_maps2 = [{"neg": neg_bf[c * RC:(c + 1) * RC]} for c in cores]
    res2 = run_bass_kernel_spmd(nc2, in_maps2, core_ids=cores)
    sneg = np.concatenate([res2.results[c]["sneg"] for c in cores],
                          axis=0).astype(np.float32)           # [B, K]

    # ---------------- host: positives (tiny) in f32
    posw = int(cnt.max())
    pos_pad = np.full((B, posw), -np.inf, dtype=np.float32)
    rows, cols = np.nonzero(mask)
    within = np.arange(rows.size) - np.repeat(
        np.concatenate([[0], np.cumsum(cnt)[:-1]]), cnt)
    pos_pad[rows, within] = cos[rows, cols]
    spos = -np.sort(-pos_pad, axis=-1)[:, :pos_min]            # [B, pos_min]

    # ---------------- host assembly (matches reference exactly)
    tk = min(top_k, pos_min)
    pos_cat = np.concatenate([spos[:, :tk], spos[:, pos_min - 1:pos_min]], axis=1)
    reps = pos_cat.shape[1]
    Tf = np.float32(T)
    pos_scaled = (pos_cat / Tf).astype(np.float32)
    neg_scaled = (sneg[:, :neg_min] / Tf).astype(np.float32)

    out = np.empty((B * reps, 1 + neg_min), dtype=np.float32)
    out3 = out.reshape(B, reps, 1 + neg_min)
    out3[:, :, 0] = pos_scaled
    out3[:, :, 1:] = neg_scaled[:, None, :]
    return out
